# revision 21
# baseline (speedup 1.0000x reference)
"""Trainium2 Bass kernel for nn_MultiHeadAttention (B=2, S=2048, D=1024, H=16).

Sharding: data-parallel over batch (2) x tensor-parallel over heads (4 groups
of 4 heads) = 8 cores. w_q/w_k/w_v column-parallel, w_o row-parallel
(Megatron); the row-parallel partial sums are reduced on the host during
unsharding.

Per-core kernel (heads h0..h3 of one batch):
  - Projections and attention are INTERLEAVED chunk-by-chunk: causal
    attention for query chunk j only needs k/v chunks 0..j, so the schedule
    is P0 A0 P1 O0 A1 P2 O1 A2 P3 O2 A3 O3 (P=chunk projection, A=attention,
    O=output projection).  This fills the PE idle gaps left by the exp-bound
    attention inner loop (ScalarE is the bottleneck there) with projection
    matmuls.
  - The kt loop is software-pipelined: scores+exp for kt+1 are emitted
    before attn@v for kt, so the in-order PE queue computes the next score
    tile while ScalarE exps the current one.
  - plain For_i carries an all-engine barrier per iteration, so the bench
    loop runs 4 bodies per For_i iteration (barrier amortized 4x, bodies
    overlap).  Output DMAs issue from the Activation HWDGE queue so they
    never head-of-line-block the next body's input prefetch on the SP
    queue.  Input chunks prefetch one attention phase ahead.
  - qT/kT/vT inputs arrive pre-transposed [D, S] in fp16 (host prep), DMA'd
    in [128, 512] chunk tiles.  Weights/masks/biases are DMA'd once outside
    the benchmark loop (loop-invariant).
  - q/k projections produce qT_h [c, s] layout; bias is added during the
    PSUM->SBUF evacuation via a per-partition tensor_scalar_add (no bias
    matmuls).  v projection produces v [s, c] natural layout with a ones
    column per head (memset once at init); its bias is added during
    evacuation with a pre-broadcast [128, C] bias tile (no bias matmuls).
  - scores are computed transposed, [sk, sq], per head-pair (the two K=64
    matmuls auto-pack into disjoint PE row groups); diagonal tiles narrow
    the score matmul, the exp, and the attn@v matmul to the unmasked
    columns; the 128-wide triangular band is multiplied by a precomputed
    0/1 mask.  exp on ScalarE (scale=1/8 folded in).
  - attn @ v_ext gives xT [dh(+1), sq] per head; row 64 is the softmax
    denominator D.  Normalization: copy the D row to a base-0 SBUF tile,
    partition_broadcast the raw D (GpSimd, SBUF->SBUF only -- PSUM-sourced
    custom DVE ops and non-zero base partitions are broken on HW),
    reciprocal_approx_fast on the broadcast, then multiply during the
    PSUM->SBUF evacuation.
  - out = xT_norm.T @ w_o per sq chunk, accumulated in fp32, written as an
    fp16 partial (halves output DMA; host reduces in fp32).  The out-proj
    for chunk j is emitted after chunk j+1's projections so its
    x-normalization chain (DVE/GpSimd) hides under projection matmuls.
  - PSUM: one shared pool (2 x [128, 1024] f32 = 4 banks) rotates between
    projection outputs and score tiles; the accumulation pool (4 x
    [128, 512] = 4 banks) holds the per-head attn@v accumulators and the
    out-proj tiles.
"""

import numpy as np

D_MODEL = 1024
NUM_HEADS = 16
HEAD_DIM = 64
B = 2
S = 2048
N_CORES = 8
HEADS_PER_CORE = 4
C = HEADS_PER_CORE * HEAD_DIM  # 256 channels per core
SQ = 512                       # sq chunk (free dim of score matmuls)
NJ = S // SQ                   # 4 sq chunks
KT = 128                       # sk tile
NKT = S // KT                  # 16 sk tiles
NDT = D_MODEL // 128           # 8 contraction tiles for projections
UNROLL = 6                     # bodies per For_i iteration (barrier amortize)
# f16 Schraudolph exp: bitcast_f16(round(1024*(s*0.125*log2e + 15 - sigma)));
# sigma tuned for zero-mean multiplicative error (RMS ~1.8%).  Used on a few
# attention(3) tiles where ScalarE is the bottleneck and DVE idles.
SCH_A = float(1024 * 1.4426950408889634 * 0.125)
SCH_B = float(1024 * (15 - 0.05640058203281127))

_CACHE = {}


def _build(loop_n=1, causal=True, unroll=False):
    import concourse.bass as bass
    import concourse.mybir as mybir
    import concourse.tile as tile
    from concourse import bacc

    dt = mybir.dt
    f16 = dt.float16
    f32 = dt.float32
    AF = mybir.ActivationFunctionType

    nc = bacc.Bacc(trn_type="TRN2", target_bir_lowering=False, debug=False)

    qT = nc.dram_tensor("qT", [D_MODEL, S], f16, kind="ExternalInput").ap()
    kT = nc.dram_tensor("kT", [D_MODEL, S], f16, kind="ExternalInput").ap()
    vT = nc.dram_tensor("vT", [D_MODEL, S], f16, kind="ExternalInput").ap()
    wq = nc.dram_tensor("wq", [D_MODEL, C], f16, kind="ExternalInput").ap()
    wk = nc.dram_tensor("wk", [D_MODEL, C], f16, kind="ExternalInput").ap()
    wv = nc.dram_tensor("wv", [D_MODEL, C], f16, kind="ExternalInput").ap()
    bq = nc.dram_tensor("bq", [128, 2], f32, kind="ExternalInput").ap()
    bk = nc.dram_tensor("bk", [128, 2], f32, kind="ExternalInput").ap()
    bv = nc.dram_tensor("bv", [1, C], f32, kind="ExternalInput").ap()
    wo = nc.dram_tensor("wo", [C, D_MODEL], f16, kind="ExternalInput").ap()
    masks = nc.dram_tensor("masks", [4, 128, 2 * SQ], f16, kind="ExternalInput").ap()
    out = nc.dram_tensor("out", [S, D_MODEL], f16, kind="ExternalOutput").ap()

    with tile.TileContext(nc) as tc:
        with tc.tile_pool(name="singles", bufs=1) as singles:
            # persistent SBUF tensors
            wq_sb = singles.tile([128, NDT, C], f16, tag="wq")
            wk_sb = singles.tile([128, NDT, C], f16, tag="wk")
            wv_sb = singles.tile([128, NDT, C], f16, tag="wv")
            wo_sb = singles.tile([128, 2, D_MODEL], f16, tag="wo")
            mask_sb = singles.tile([128, 4, 2 * SQ], f16, tag="mask")
            bq_sb = singles.tile([128, 2], f32, tag="bq")
            bk_sb = singles.tile([128, 2], f32, tag="bk")
            bv_sb = singles.tile([1, C], f32, tag="bv")
            bvbc_sb = singles.tile([128, C], f32, tag="bvbc")
            q_sb = singles.tile([128, 2, S], f16, tag="q")     # [c, s] 2 part tiles
            k_sb = singles.tile([128, 2, S], f16, tag="k")
            x_sb = singles.tile([128, 2, S], f16, tag="x")     # normalized attn out, [c, s]
            v_sb = singles.tile([128, NKT, HEADS_PER_CORE * 65], f16, tag="v")

            # loop-invariant init: v ones-columns, weights, masks, biases
            nc.vector.memset(v_sb[:], 1.0)
            nc.sync.dma_start(out=bq_sb[:], in_=bq)
            nc.sync.dma_start(out=bk_sb[:], in_=bk)
            nc.sync.dma_start(out=bv_sb[:], in_=bv)
            nc.gpsimd.partition_broadcast(bvbc_sb[:], bv_sb[:])
            for ld in range(NDT):
                nc.sync.dma_start(out=wq_sb[:, ld, :], in_=wq[ld * 128:(ld + 1) * 128, :])
                nc.sync.dma_start(out=wk_sb[:, ld, :], in_=wk[ld * 128:(ld + 1) * 128, :])
                nc.sync.dma_start(out=wv_sb[:, ld, :], in_=wv[ld * 128:(ld + 1) * 128, :])
            for t in range(4):
                nc.sync.dma_start(out=mask_sb[:, t, :], in_=masks[t])
            for ct in range(2):
                nc.sync.dma_start(out=wo_sb[:, ct, :], in_=wo[ct * 128:(ct + 1) * 128, :])

            def body(inp, scp, accp, esb, nrm, osb):
                def fetch_chunk(j):
                    """DMA the [128, 512] input tiles for chunk j; returns
                    {src: [8 tiles]}."""
                    sl = slice(j * SQ, (j + 1) * SQ)
                    tiles = {}
                    for nm, src in (("q", qT), ("k", kT), ("v", vT)):
                        tl = []
                        for ld in range(NDT):
                            it = inp.tile([128, SQ], f16, tag="ink")
                            nc.sync.dma_start(
                                out=it[:], in_=src[ld * 128:(ld + 1) * 128, sl]
                            )
                            tl.append(it)
                        tiles[nm] = tl
                    return tiles

                def proj_units(j, ink):
                    """Chunk-j projection as 8 independent PE work units
                    (each allocates its own score-pool tile), drainable
                    between attention kt iterations."""
                    sl = slice(j * SQ, (j + 1) * SQ)
                    units = []
                    for nm, w_sb, b_sb, dest in (
                        ("q", wq_sb, bq_sb, q_sb),
                        ("k", wk_sb, bk_sb, k_sb),
                    ):
                        for ct in range(2):
                            def u(nm=nm, w_sb=w_sb, b_sb=b_sb, dest=dest, ct=ct):
                                ps = scp.tile([128, 2 * SQ], f32, tag="sc")
                                for ld in range(NDT):
                                    nc.tensor.matmul(
                                        ps[:, 0:SQ],
                                        w_sb[:, ld, ct * 128:(ct + 1) * 128],
                                        ink[nm][ld][:],
                                        start=(ld == 0), stop=(ld == NDT - 1),
                                    )
                                nc.vector.tensor_scalar_add(
                                    dest[:, ct, sl], ps[:, 0:SQ],
                                    b_sb[:, ct:ct + 1],
                                )
                            units.append(u)
                    # v projection: natural [s, c], bias during evacuation
                    for sti in range(4):
                        def u(sti=sti):
                            pv = scp.tile([128, 2 * SQ], f32, tag="sc")
                            pvq = pv[:, 0:C]
                            for ld in range(NDT):
                                nc.tensor.matmul(
                                    pvq,
                                    ink["v"][ld][:, sti * 128:(sti + 1) * 128],
                                    wv_sb[:, ld, :], start=(ld == 0),
                                    stop=(ld == NDT - 1),
                                )
                            st = 4 * j + sti
                            vdst = v_sb[:, st, :].rearrange(
                                "p (h e) -> p h e", e=65)
                            nc.vector.tensor_add(
                                vdst[:, :, 0:64],
                                pvq.rearrange("p (h e) -> p h e", e=64),
                                bvbc_sb[:].rearrange("p (h e) -> p h e", e=64),
                            )
                        units.append(u)
                    return units

                def attention(j, oput=(), fillers=None):
                    """oput: out-proj units (use the hp=1 accumulator tags) --
                    MUST all drain inside hp=0's kt loop.  fillers: deque of
                    projection units drained one per kt slot."""
                    nkt = 4 * (j + 1) if causal else NKT
                    oput = list(oput)
                    assert len(oput) <= nkt

                    def sc_exp(hp, kt):
                        t = kt - 4 * j if (causal and kt >= 4 * j) else 0
                        diag = causal and kt >= 4 * j
                        psc = scp.tile([128, 2 * SQ], f32, tag="sc")
                        for hi in range(2):
                            lhsT = k_sb[64 * hi:64 * hi + 64, hp,
                                        kt * 128:(kt + 1) * 128]
                            rhs = q_sb[64 * hi:64 * hi + 64, hp,
                                       j * SQ + t * 128:(j + 1) * SQ]
                            nc.tensor.matmul(
                                psc[:, hi * SQ + t * 128:(hi + 1) * SQ],
                                lhsT, rhs, start=True, stop=True,
                            )
                        if False:  # Schraudolph exp offload: no net win (DVE-queue interference)
                            # ScalarE-relief: approximate exp on DVE via the
                            # f16 Schraudolph int trick (validated vs exact;
                            # softmax cancels the zero-mean error).
                            e2i = esb.tile([128, 2 * SQ], dt.int16, tag="e2i")
                            nc.vector.tensor_scalar(
                                e2i[:], psc[:], SCH_A, SCH_B,
                                mybir.AluOpType.mult, mybir.AluOpType.add,
                            )
                            return e2i, t, True
                        e2 = esb.tile([128, 2 * SQ], f16, tag="e2")
                        if diag:
                            # diagonal tile: columns < t*128 are fully masked
                            # -> narrowed; the 128-wide band at t*128 is
                            # triangular (exp + mask).
                            e3 = e2[:].rearrange("p (h c) -> p h c", h=2)
                            p3 = psc[:].rearrange("p (h c) -> p h c", h=2)
                            nc.scalar.activation(
                                e3[:, :, t * 128:SQ], p3[:, :, t * 128:SQ],
                                AF.Exp, scale=0.125,
                            )
                            m3 = mask_sb[:, t, :].rearrange(
                                "p (h c) -> p h c", h=2)
                            nc.vector.tensor_mul(
                                e3[:, :, t * 128:(t + 1) * 128],
                                e3[:, :, t * 128:(t + 1) * 128],
                                m3[:, :, t * 128:(t + 1) * 128],
                            )
                        else:
                            nc.scalar.activation(e2[:], psc[:], AF.Exp,
                                                 scale=0.125)
                        return e2, t, False

                    for hp in range(2):
                        po = {}
                        for hi in range(2):
                            po[hi] = accp.tile(
                                [128, SQ], f32, tag=f"acc{2 * hp + hi}",
                                name=f"po{hp}{hi}"
                            )
                        # software-pipelined kt loop: scores/exp run one tile
                        # ahead of attn@v on the in-order PE queue; filler
                        # units slot in behind each attn@v.
                        pend = sc_exp(hp, 0)
                        for kt in range(nkt):
                            cur, pend = pend, (sc_exp(hp, kt + 1)
                                               if kt + 1 < nkt else None)
                            e2, t, is_i16 = cur
                            for hi in range(2):
                                h = 2 * hp + hi
                                lhsT = v_sb[:, kt, h * 65:(h + 1) * 65]
                                rhs = e2[:, hi * SQ + t * 128:(hi + 1) * SQ]
                                if is_i16:
                                    rhs = rhs.bitcast(f16)
                                nc.tensor.matmul(
                                    po[hi][0:65, t * 128:SQ], lhsT, rhs,
                                    start=(kt == 0), stop=(kt == nkt - 1),
                                )
                            if hp == 0 and kt < len(oput):
                                oput[kt]()
                            elif fillers:
                                fillers.popleft()()
                        for hi in range(2):
                            # D row -> SBUF (base partition 0), broadcast raw
                            # D (GpSimd), reciprocal on the broadcast, then
                            # scale during the PSUM->SBUF evacuation.
                            dsb = nrm.tile([1, SQ], f32, tag="dsb",
                                           name=f"d{hp}{hi}")
                            nc.vector.tensor_copy(dsb[:], po[hi][64:65, :])
                            dbc = nrm.tile([64, SQ], f32, tag="dbc",
                                           name=f"db{hp}{hi}")
                            nc.gpsimd.partition_broadcast(dbc[:], dsb[:])
                            rbc = nrm.tile([64, SQ], f32, tag="rbc",
                                           name=f"rb{hp}{hi}")
                            nc.vector.reciprocal_approx_fast(rbc[:], dbc[:])
                            nc.vector.tensor_mul(
                                x_sb[64 * hi:64 * hi + 64, hp,
                                     j * SQ:(j + 1) * SQ],
                                po[hi][0:64, :], rbc[:],
                            )

                def outproj_units(j, last=False):
                    """Out-proj for chunk j as 4 t-tile units.  pf tiles use
                    the hp=1 accumulator tags (acc2/acc3), free during the
                    next attention's hp=0 kt loop."""
                    units = []
                    for t in range(4 * j, 4 * j + 4):
                        def u(t=t):
                            pf = [accp.tile([128, SQ], f32, tag=f"acc{2 + n}",
                                            name=f"pf{n}") for n in range(2)]
                            for ct in range(2):
                                lhsT = x_sb[:, ct, t * 128:(t + 1) * 128]
                                for n in range(2):
                                    nc.tensor.matmul(
                                        pf[n][:], lhsT,
                                        wo_sb[:, ct, n * SQ:(n + 1) * SQ],
                                        start=(ct == 0), stop=(ct == 1),
                                    )
                            ot = osb.tile([128, D_MODEL], f16, tag="ot")
                            for n in range(2):
                                nc.vector.tensor_copy(
                                    ot[:, n * SQ:(n + 1) * SQ], pf[n][:])
                            # The last out-proj's DMAs issue from the
                            # Activation HWDGE queue (idle at body end) so
                            # they never head-of-line-block the next body's
                            # input prefetch on the SP queue.
                            eng = nc.scalar if last else nc.sync
                            eng.dma_start(out=out[t * 128:(t + 1) * 128, :],
                                          in_=ot[:])
                        units.append(u)
                    return units

                from collections import deque
                if causal:
                    ink = fetch_chunk(0)
                    for u in proj_units(0, ink):
                        u()
                    for j in range(NJ):
                        fillers = deque()
                        if j + 1 < NJ:
                            fillers.extend(proj_units(j + 1, fetch_chunk(j + 1)))
                        oput = outproj_units(j - 1) if j > 0 else ()
                        attention(j, oput=oput, fillers=fillers)
                        while fillers:
                            fillers.popleft()()
                    for u in outproj_units(NJ - 1, last=True):
                        u()
                else:
                    for j in range(NJ):
                        for u in proj_units(j, fetch_chunk(j)):
                            u()
                    for j in range(NJ):
                        attention(j)
                        for u in outproj_units(j, last=(j == NJ - 1)):
                            u()

            with (
                tc.tile_pool(name="inp", bufs=36) as inp,
                tc.tile_pool(name="sc", bufs=2, space="PSUM") as scp,
                tc.tile_pool(name="acc", bufs=1, space="PSUM") as accp,
                tc.tile_pool(name="esb", bufs=8) as esb,
                tc.tile_pool(name="nrm", bufs=8) as nrm,
                tc.tile_pool(name="osb", bufs=4) as osb,
            ):
                pools = (inp, scp, accp, esb, nrm, osb)
                if loop_n == 1:
                    body(*pools)
                elif unroll:
                    for _ in range(loop_n):
                        body(*pools)
                else:
                    hint = (
                        mybir.EngineType.PE, mybir.EngineType.DVE,
                        mybir.EngineType.Activation, mybir.EngineType.Pool,
                        mybir.EngineType.SP,
                    )
                    nfull, rem = divmod(loop_n, UNROLL)
                    if nfull:
                        with tc.For_i(0, nfull, 1, hint_engines=hint):
                            for _ in range(UNROLL):
                                body(*pools)
                    for _ in range(rem):
                        body(*pools)

    nc.compile()
    return nc


def _masks_np():
    m = np.zeros((4, 128, 2 * SQ), np.float16)
    r = np.arange(128)[:, None]
    c = np.arange(SQ)[None, :]
    for t in range(4):
        allow = (t * 128 + r) <= c
        m[t, :, 0:SQ] = allow
        m[t, :, SQ:2 * SQ] = allow
    return m


def _in_maps(query, key, value, w_q, b_q, w_k, b_k, w_v, b_v, w_o):
    f16 = np.float16
    masks = _masks_np()
    per_batch = []
    for b in range(B):
        per_batch.append((
            query[b].T.astype(f16),
            key[b].T.astype(f16),
            value[b].T.astype(f16),
        ))
    in_maps = []
    for core in range(N_CORES):
        b, g = divmod(core, N_CORES // B)
        cols = slice(g * C, (g + 1) * C)
        qTb, kTb, vTb = per_batch[b]
        in_maps.append({
            "qT": qTb, "kT": kTb, "vT": vTb,
            "wq": w_q[:, cols].astype(f16),
            "wk": w_k[:, cols].astype(f16),
            "wv": w_v[:, cols].astype(f16),
            # [128, 2] layout: bq2[p, ct] = b_q[g*C + ct*128 + p]
            "bq": b_q[cols].reshape(2, 128).T.copy().astype(np.float32),
            "bk": b_k[cols].reshape(2, 128).T.copy().astype(np.float32),
            "bv": b_v[cols].reshape(1, -1).astype(np.float32),
            "wo": np.ascontiguousarray(w_o[cols, :]).astype(f16),
            "masks": masks,
        })
    return in_maps


def kernel(query, key, value, mask, w_q, b_q, w_k, b_k, w_v, b_v, w_o, b_o):
    from concourse.bass_utils import run_bass_kernel_spmd

    query = np.asarray(query, np.float32)
    key = np.asarray(key, np.float32)
    value = np.asarray(value, np.float32)
    causal = bool(np.asarray(mask).reshape(-1)[0]) if np.asarray(mask).size else False

    ck = ("prog", causal)
    if ck not in _CACHE:
        _CACHE[ck] = _build(loop_n=1, causal=causal)
    nc = _CACHE[ck]

    in_maps = _in_maps(query, key, value,
                       np.asarray(w_q, np.float32), np.asarray(b_q, np.float32),
                       np.asarray(w_k, np.float32), np.asarray(b_k, np.float32),
                       np.asarray(w_v, np.float32), np.asarray(b_v, np.float32),
                       np.asarray(w_o, np.float32))
    res = run_bass_kernel_spmd(nc, in_maps, core_ids=list(range(N_CORES)))
    outs = [res.results[c]["out"] for c in range(N_CORES)]
    gpb = N_CORES // B
    full = np.empty((B, S, D_MODEL), np.float32)
    bo = np.asarray(b_o, np.float32)
    for b in range(B):
        acc = outs[gpb * b].astype(np.float32)
        for g in range(1, gpb):
            acc += outs[gpb * b + g].astype(np.float32)
        full[b] = acc + bo[None, :]
    return full


# revision 22
# speedup vs baseline: 1.1746x; 1.1746x over previous
"""Trainium2 Bass kernel for nn_MultiHeadAttention (B=2, S=2048, D=1024, H=16).

Sharding: data-parallel over batch (2) x tensor-parallel over heads (4 groups
of 4 heads) = 8 cores. w_q/w_k/w_v column-parallel, w_o row-parallel
(Megatron); the row-parallel partial sums are reduced on the host during
unsharding.

Per-core kernel (heads h0..h3 of one batch):
  - Projections and attention are INTERLEAVED chunk-by-chunk: causal
    attention for query chunk j only needs k/v chunks 0..j, so the schedule
    is P0 A0 P1 O0 A1 P2 O1 A2 P3 O2 A3 O3 (P=chunk projection, A=attention,
    O=output projection).  This fills the PE idle gaps left by the exp-bound
    attention inner loop (ScalarE is the bottleneck there) with projection
    matmuls.
  - The kt loop is software-pipelined: scores+exp for kt+1 are emitted
    before attn@v for kt, so the in-order PE queue computes the next score
    tile while ScalarE exps the current one.
  - plain For_i carries an all-engine barrier per iteration, so the bench
    loop runs 4 bodies per For_i iteration (barrier amortized 4x, bodies
    overlap).  Output DMAs issue from the Activation HWDGE queue so they
    never head-of-line-block the next body's input prefetch on the SP
    queue.  Input chunks prefetch one attention phase ahead.
  - qT/kT/vT inputs arrive pre-transposed [D, S] in fp16 (host prep), DMA'd
    in [128, 512] chunk tiles.  Weights/masks/biases are DMA'd once outside
    the benchmark loop (loop-invariant).
  - q/k projections produce qT_h [c, s] layout; bias is added during the
    PSUM->SBUF evacuation via a per-partition tensor_scalar_add (no bias
    matmuls).  v projection produces v [s, c] natural layout with a ones
    column per head (memset once at init); its bias is added during
    evacuation with a pre-broadcast [128, C] bias tile (no bias matmuls).
  - scores are computed transposed, [sk, sq], per head-pair (the two K=64
    matmuls auto-pack into disjoint PE row groups); diagonal tiles narrow
    the score matmul, the exp, and the attn@v matmul to the unmasked
    columns; the 128-wide triangular band is multiplied by a precomputed
    0/1 mask.  exp on ScalarE (scale=1/8 folded in).
  - attn @ v_ext gives xT [dh(+1), sq] per head; row 64 is the softmax
    denominator D.  Normalization: copy the D row to a base-0 SBUF tile,
    partition_broadcast the raw D (GpSimd, SBUF->SBUF only -- PSUM-sourced
    custom DVE ops and non-zero base partitions are broken on HW),
    reciprocal_approx_fast on the broadcast, then multiply during the
    PSUM->SBUF evacuation.
  - out = xT_norm.T @ w_o per sq chunk, accumulated in fp32, written as an
    fp16 partial (halves output DMA; host reduces in fp32).  The out-proj
    for chunk j is emitted after chunk j+1's projections so its
    x-normalization chain (DVE/GpSimd) hides under projection matmuls.
  - PSUM: one shared pool (2 x [128, 1024] f32 = 4 banks) rotates between
    projection outputs and score tiles; the accumulation pool (4 x
    [128, 512] = 4 banks) holds the per-head attn@v accumulators and the
    out-proj tiles.
"""

import numpy as np

D_MODEL = 1024
NUM_HEADS = 16
HEAD_DIM = 64
B = 2
S = 2048
N_CORES = 8
HEADS_PER_CORE = 4
C = HEADS_PER_CORE * HEAD_DIM  # 256 channels per core
SQ = 512                       # sq chunk (free dim of score matmuls)
NJ = S // SQ                   # 4 sq chunks
KT = 128                       # sk tile
NKT = S // KT                  # 16 sk tiles
NDT = D_MODEL // 128           # 8 contraction tiles for projections
UNROLL = 4                     # bodies per For_i iteration (barrier amortize)

_CACHE = {}


def _build(loop_n=1, causal=True, unroll=False):
    import concourse.bass as bass
    import concourse.mybir as mybir
    import concourse.tile as tile
    from concourse import bacc

    dt = mybir.dt
    f16 = dt.float16
    f32 = dt.float32
    AF = mybir.ActivationFunctionType

    nc = bacc.Bacc(trn_type="TRN2", target_bir_lowering=False, debug=False)

    qT = nc.dram_tensor("qT", [D_MODEL, S], f16, kind="ExternalInput").ap()
    kT = nc.dram_tensor("kT", [D_MODEL, S], f16, kind="ExternalInput").ap()
    vT = nc.dram_tensor("vT", [D_MODEL, S], f16, kind="ExternalInput").ap()
    wq = nc.dram_tensor("wq", [D_MODEL, C], f16, kind="ExternalInput").ap()
    wk = nc.dram_tensor("wk", [D_MODEL, C], f16, kind="ExternalInput").ap()
    wv = nc.dram_tensor("wv", [D_MODEL, C], f16, kind="ExternalInput").ap()
    bq = nc.dram_tensor("bq", [128, 2], f32, kind="ExternalInput").ap()
    bk = nc.dram_tensor("bk", [128, 2], f32, kind="ExternalInput").ap()
    bv = nc.dram_tensor("bv", [1, C], f32, kind="ExternalInput").ap()
    wo = nc.dram_tensor("wo", [C, D_MODEL], f16, kind="ExternalInput").ap()
    masks = nc.dram_tensor("masks", [4, 128, 2 * SQ], f16, kind="ExternalInput").ap()
    out = nc.dram_tensor("out", [S, D_MODEL], f16, kind="ExternalOutput").ap()

    with tile.TileContext(nc) as tc:
        with tc.tile_pool(name="singles", bufs=1) as singles:
            # persistent SBUF tensors
            wq_sb = singles.tile([128, NDT, C], f16, tag="wq")
            wk_sb = singles.tile([128, NDT, C], f16, tag="wk")
            wv_sb = singles.tile([128, NDT, C], f16, tag="wv")
            wo_sb = singles.tile([128, 2, D_MODEL], f16, tag="wo")
            mask_sb = singles.tile([128, 4, 2 * SQ], f16, tag="mask")
            bq_sb = singles.tile([128, 2], f32, tag="bq")
            bk_sb = singles.tile([128, 2], f32, tag="bk")
            bv_sb = singles.tile([1, C], f32, tag="bv")
            bvbc_sb = singles.tile([128, C], f32, tag="bvbc")
            q_sb = singles.tile([128, 2, S], f16, tag="q")     # [c, s] 2 part tiles
            k_sb = singles.tile([128, 2, S], f16, tag="k")
            x_sb = singles.tile([128, 2, S], f16, tag="x")     # normalized attn out, [c, s]
            v_sb = singles.tile([128, NKT, HEADS_PER_CORE * 65], f16, tag="v")

            # loop-invariant init: v ones-columns, weights, masks, biases
            nc.vector.memset(v_sb[:], 1.0)
            nc.sync.dma_start(out=bq_sb[:], in_=bq)
            nc.sync.dma_start(out=bk_sb[:], in_=bk)
            nc.sync.dma_start(out=bv_sb[:], in_=bv)
            nc.gpsimd.partition_broadcast(bvbc_sb[:], bv_sb[:])
            for ld in range(NDT):
                nc.sync.dma_start(out=wq_sb[:, ld, :], in_=wq[ld * 128:(ld + 1) * 128, :])
                nc.sync.dma_start(out=wk_sb[:, ld, :], in_=wk[ld * 128:(ld + 1) * 128, :])
                nc.sync.dma_start(out=wv_sb[:, ld, :], in_=wv[ld * 128:(ld + 1) * 128, :])
            for t in range(4):
                nc.sync.dma_start(out=mask_sb[:, t, :], in_=masks[t])
            for ct in range(2):
                nc.sync.dma_start(out=wo_sb[:, ct, :], in_=wo[ct * 128:(ct + 1) * 128, :])

            def body(inp, scp, accp, esb, nrm, osb):
                def fetch_chunk(j):
                    """DMA the [128, 512] input tiles for chunk j; returns
                    {src: [8 tiles]}."""
                    sl = slice(j * SQ, (j + 1) * SQ)
                    tiles = {}
                    for nm, src in (("q", qT), ("k", kT), ("v", vT)):
                        tl = []
                        for ld in range(NDT):
                            it = inp.tile([128, SQ], f16, tag="ink")
                            nc.sync.dma_start(
                                out=it[:], in_=src[ld * 128:(ld + 1) * 128, sl]
                            )
                            tl.append(it)
                        tiles[nm] = tl
                    return tiles

                def proj_chunk(j, ink):
                    sl = slice(j * SQ, (j + 1) * SQ)
                    # q/k projections for chunk j -> qT_h/kT_h [c, s] slices
                    for nm, w_sb, b_sb, dest in (
                        ("q", wq_sb, bq_sb, q_sb),
                        ("k", wk_sb, bk_sb, k_sb),
                    ):
                        ps = scp.tile([128, 2 * SQ], f32, tag="sc")
                        for ld in range(NDT):
                            for ct in range(2):
                                nc.tensor.matmul(
                                    ps[:, ct * SQ:(ct + 1) * SQ],
                                    w_sb[:, ld, ct * 128:(ct + 1) * 128],
                                    ink[nm][ld][:],
                                    start=(ld == 0), stop=(ld == NDT - 1),
                                )
                        for ct in range(2):
                            nc.vector.tensor_scalar_add(
                                dest[:, ct, sl], ps[:, ct * SQ:(ct + 1) * SQ],
                                b_sb[:, ct:ct + 1],
                            )
                    # v projection for chunk j: natural [s, c], bias during evac
                    pv = scp.tile([128, 2 * SQ], f32, tag="sc")
                    for sti in range(4):
                        pvq = pv[:, sti * C:(sti + 1) * C]
                        for ld in range(NDT):
                            nc.tensor.matmul(
                                pvq, ink["v"][ld][:, sti * 128:(sti + 1) * 128],
                                wv_sb[:, ld, :], start=(ld == 0),
                                stop=(ld == NDT - 1),
                            )
                    for sti in range(4):
                        st = 4 * j + sti
                        vdst = v_sb[:, st, :].rearrange("p (h e) -> p h e", e=65)
                        nc.vector.tensor_add(
                            vdst[:, :, 0:64],
                            pv[:, sti * C:(sti + 1) * C].rearrange(
                                "p (h e) -> p h e", e=64),
                            bvbc_sb[:].rearrange("p (h e) -> p h e", e=64),
                        )

                def attention(j):
                    nkt = 4 * (j + 1) if causal else NKT

                    def sc_exp(hp, kt):
                        t = kt - 4 * j if (causal and kt >= 4 * j) else 0
                        diag = causal and kt >= 4 * j
                        psc = scp.tile([128, 2 * SQ], f32, tag="sc")
                        for hi in range(2):
                            lhsT = k_sb[64 * hi:64 * hi + 64, hp,
                                        kt * 128:(kt + 1) * 128]
                            rhs = q_sb[64 * hi:64 * hi + 64, hp,
                                       j * SQ + t * 128:(j + 1) * SQ]
                            nc.tensor.matmul(
                                psc[:, hi * SQ + t * 128:(hi + 1) * SQ],
                                lhsT, rhs, start=True, stop=True,
                            )
                        e2 = esb.tile([128, 2 * SQ], f16, tag="e2")
                        if diag:
                            # diagonal tile: columns < t*128 are fully masked
                            # -> narrowed; the 128-wide band at t*128 is
                            # triangular (exp + mask).
                            e3 = e2[:].rearrange("p (h c) -> p h c", h=2)
                            p3 = psc[:].rearrange("p (h c) -> p h c", h=2)
                            nc.scalar.activation(
                                e3[:, :, t * 128:SQ], p3[:, :, t * 128:SQ],
                                AF.Exp, scale=0.125,
                            )
                            m3 = mask_sb[:, t, :].rearrange(
                                "p (h c) -> p h c", h=2)
                            nc.vector.tensor_mul(
                                e3[:, :, t * 128:(t + 1) * 128],
                                e3[:, :, t * 128:(t + 1) * 128],
                                m3[:, :, t * 128:(t + 1) * 128],
                            )
                        else:
                            nc.scalar.activation(e2[:], psc[:], AF.Exp,
                                                 scale=0.125)
                        return e2, t

                    for hp in range(2):
                        po = {}
                        for hi in range(2):
                            po[hi] = accp.tile(
                                [128, SQ], f32, tag=f"acc{hi}", name=f"po{hp}{hi}"
                            )
                        # software-pipelined kt loop: scores/exp run one tile
                        # ahead of attn@v on the in-order PE queue.
                        pend = sc_exp(hp, 0)
                        for kt in range(nkt):
                            cur, pend = pend, (sc_exp(hp, kt + 1)
                                               if kt + 1 < nkt else None)
                            e2, t = cur
                            for hi in range(2):
                                h = 2 * hp + hi
                                lhsT = v_sb[:, kt, h * 65:(h + 1) * 65]
                                nc.tensor.matmul(
                                    po[hi][0:65, t * 128:SQ], lhsT,
                                    e2[:, hi * SQ + t * 128:(hi + 1) * SQ],
                                    start=(kt == 0), stop=(kt == nkt - 1),
                                )
                        for hi in range(2):
                            # D row -> SBUF (base partition 0), broadcast raw
                            # D (GpSimd), reciprocal on the broadcast, then
                            # scale during the PSUM->SBUF evacuation.
                            dsb = nrm.tile([1, SQ], f32, tag="dsb",
                                           name=f"d{hp}{hi}")
                            nc.vector.tensor_copy(dsb[:], po[hi][64:65, :])
                            dbc = nrm.tile([64, SQ], f32, tag="dbc",
                                           name=f"db{hp}{hi}")
                            nc.gpsimd.partition_broadcast(dbc[:], dsb[:])
                            rbc = nrm.tile([64, SQ], f32, tag="rbc",
                                           name=f"rb{hp}{hi}")
                            nc.vector.reciprocal_approx_fast(rbc[:], dbc[:])
                            nc.vector.tensor_mul(
                                x_sb[64 * hi:64 * hi + 64, hp,
                                     j * SQ:(j + 1) * SQ],
                                po[hi][0:64, :], rbc[:],
                            )

                def outproj(j, last=False):
                    for t in range(4 * j, 4 * j + 4):
                        pf = [accp.tile([128, SQ], f32, tag=f"acc{n}",
                                        name=f"pf{n}") for n in range(2)]
                        for ct in range(2):
                            lhsT = x_sb[:, ct, t * 128:(t + 1) * 128]
                            for n in range(2):
                                nc.tensor.matmul(
                                    pf[n][:], lhsT,
                                    wo_sb[:, ct, n * SQ:(n + 1) * SQ],
                                    start=(ct == 0), stop=(ct == 1),
                                )
                        ot = osb.tile([128, D_MODEL], f16, tag="ot")
                        for n in range(2):
                            nc.vector.tensor_copy(ot[:, n * SQ:(n + 1) * SQ],
                                                  pf[n][:])
                        # The last out-proj's DMAs issue from the Activation
                        # HWDGE queue (idle at body end) so they never
                        # head-of-line-block the next body's input prefetch
                        # on the SP queue.
                        eng = nc.scalar if last else nc.sync
                        eng.dma_start(out=out[t * 128:(t + 1) * 128, :],
                                      in_=ot[:])

                if causal:
                    ink = fetch_chunk(0)
                    for j in range(NJ):
                        proj_chunk(j, ink)
                        # prefetch chunk j+1 BEFORE the out-proj DMAs so input
                        # prefetch is never queued behind output traffic.
                        if j + 1 < NJ:
                            ink = fetch_chunk(j + 1)
                        if j > 0:
                            outproj(j - 1)
                        attention(j)
                    outproj(NJ - 1, last=True)
                else:
                    for j in range(NJ):
                        proj_chunk(j, fetch_chunk(j))
                    for j in range(NJ):
                        attention(j)
                        outproj(j, last=(j == NJ - 1))

            with (
                tc.tile_pool(name="inp", bufs=36) as inp,
                tc.tile_pool(name="sc", bufs=2, space="PSUM") as scp,
                tc.tile_pool(name="acc", bufs=2, space="PSUM") as accp,
                tc.tile_pool(name="esb", bufs=8) as esb,
                tc.tile_pool(name="nrm", bufs=8) as nrm,
                tc.tile_pool(name="osb", bufs=4) as osb,
            ):
                pools = (inp, scp, accp, esb, nrm, osb)
                if loop_n == 1:
                    body(*pools)
                elif unroll:
                    for _ in range(loop_n):
                        body(*pools)
                else:
                    hint = (
                        mybir.EngineType.PE, mybir.EngineType.DVE,
                        mybir.EngineType.Activation, mybir.EngineType.Pool,
                        mybir.EngineType.SP,
                    )
                    nfull, rem = divmod(loop_n, UNROLL)
                    if nfull:
                        with tc.For_i(0, nfull, 1, hint_engines=hint):
                            for _ in range(UNROLL):
                                body(*pools)
                    for _ in range(rem):
                        body(*pools)

    nc.compile()
    return nc


def _masks_np():
    m = np.zeros((4, 128, 2 * SQ), np.float16)
    r = np.arange(128)[:, None]
    c = np.arange(SQ)[None, :]
    for t in range(4):
        allow = (t * 128 + r) <= c
        m[t, :, 0:SQ] = allow
        m[t, :, SQ:2 * SQ] = allow
    return m


def _in_maps(query, key, value, w_q, b_q, w_k, b_k, w_v, b_v, w_o):
    f16 = np.float16
    masks = _masks_np()
    per_batch = []
    for b in range(B):
        per_batch.append((
            query[b].T.astype(f16),
            key[b].T.astype(f16),
            value[b].T.astype(f16),
        ))
    in_maps = []
    for core in range(N_CORES):
        b, g = divmod(core, N_CORES // B)
        cols = slice(g * C, (g + 1) * C)
        qTb, kTb, vTb = per_batch[b]
        in_maps.append({
            "qT": qTb, "kT": kTb, "vT": vTb,
            "wq": w_q[:, cols].astype(f16),
            "wk": w_k[:, cols].astype(f16),
            "wv": w_v[:, cols].astype(f16),
            # [128, 2] layout: bq2[p, ct] = b_q[g*C + ct*128 + p]
            "bq": b_q[cols].reshape(2, 128).T.copy().astype(np.float32),
            "bk": b_k[cols].reshape(2, 128).T.copy().astype(np.float32),
            "bv": b_v[cols].reshape(1, -1).astype(np.float32),
            "wo": np.ascontiguousarray(w_o[cols, :]).astype(f16),
            "masks": masks,
        })
    return in_maps


def kernel(query, key, value, mask, w_q, b_q, w_k, b_k, w_v, b_v, w_o, b_o):
    from concourse.bass_utils import run_bass_kernel_spmd

    query = np.asarray(query, np.float32)
    key = np.asarray(key, np.float32)
    value = np.asarray(value, np.float32)
    causal = bool(np.asarray(mask).reshape(-1)[0]) if np.asarray(mask).size else False

    ck = ("prog", causal)
    if ck not in _CACHE:
        _CACHE[ck] = _build(loop_n=1, causal=causal)
    nc = _CACHE[ck]

    in_maps = _in_maps(query, key, value,
                       np.asarray(w_q, np.float32), np.asarray(b_q, np.float32),
                       np.asarray(w_k, np.float32), np.asarray(b_k, np.float32),
                       np.asarray(w_v, np.float32), np.asarray(b_v, np.float32),
                       np.asarray(w_o, np.float32))
    res = run_bass_kernel_spmd(nc, in_maps, core_ids=list(range(N_CORES)))
    outs = [res.results[c]["out"] for c in range(N_CORES)]
    gpb = N_CORES // B
    full = np.empty((B, S, D_MODEL), np.float32)
    bo = np.asarray(b_o, np.float32)
    for b in range(B):
        acc = outs[gpb * b].astype(np.float32)
        for g in range(1, gpb):
            acc += outs[gpb * b + g].astype(np.float32)
        full[b] = acc + bo[None, :]
    return full


# revision 30
# speedup vs baseline: 1.3546x; 1.1533x over previous
"""Trainium2 Bass kernel for nn_MultiHeadAttention (B=2, S=2048, D=1024, H=16).

Sharding: data-parallel over batch (2) x tensor-parallel over heads (4 groups
of 4 heads) = 8 cores. w_q/w_k/w_v column-parallel, w_o row-parallel
(Megatron); the row-parallel partial sums are reduced on the host during
unsharding.

Per-core kernel (heads h0..h3 of one batch):
  - Projections and attention are INTERLEAVED chunk-by-chunk: causal
    attention for query chunk j only needs k/v chunks 0..j, so the schedule
    is P0 A0 P1 O0 A1 P2 O1 A2 P3 O2 A3 O3 (P=chunk projection, A=attention,
    O=output projection).  This fills the PE idle gaps left by the exp-bound
    attention inner loop (ScalarE is the bottleneck there) with projection
    matmuls.
  - The kt loop is software-pipelined: scores+exp for kt+1 are emitted
    before attn@v for kt, so the in-order PE queue computes the next score
    tile while ScalarE exps the current one.
  - plain For_i carries an all-engine barrier per iteration, so the bench
    loop runs 4 bodies per For_i iteration (barrier amortized 4x, bodies
    overlap).  Output DMAs issue from the Activation HWDGE queue so they
    never head-of-line-block the next body's input prefetch on the SP
    queue.  Input chunks prefetch one attention phase ahead.
  - qT/kT/vT inputs arrive pre-transposed [D, S] in fp16 (host prep), DMA'd
    in [128, 512] chunk tiles.  Weights/masks/biases are DMA'd once outside
    the benchmark loop (loop-invariant).
  - q/k projections produce qT_h [c, s] layout; bias is added during the
    PSUM->SBUF evacuation via a per-partition tensor_scalar_add (no bias
    matmuls).  v projection produces v [s, c] natural layout with a ones
    column per head (memset once at init); its bias is added during
    evacuation with a pre-broadcast [128, C] bias tile (no bias matmuls).
  - scores are computed transposed, [sk, sq], per head-pair (the two K=64
    matmuls auto-pack into disjoint PE row groups); diagonal tiles narrow
    the score matmul, the exp, and the attn@v matmul to the unmasked
    columns; the 128-wide triangular band is multiplied by a precomputed
    0/1 mask.  exp on ScalarE (scale=1/8 folded in).
  - attn @ v_ext gives xT [dh(+1), sq] per head; row 64 is the softmax
    denominator D.  Normalization: copy the D row to a base-0 SBUF tile,
    partition_broadcast the raw D (GpSimd, SBUF->SBUF only -- PSUM-sourced
    custom DVE ops and non-zero base partitions are broken on HW),
    reciprocal_approx_fast on the broadcast, then multiply during the
    PSUM->SBUF evacuation.
  - out = xT_norm.T @ w_o per sq chunk, accumulated in fp32, written as an
    fp16 partial (halves output DMA; host reduces in fp32).  The out-proj
    for chunk j is emitted after chunk j+1's projections so its
    x-normalization chain (DVE/GpSimd) hides under projection matmuls.
  - PSUM: one shared pool (2 x [128, 1024] f32 = 4 banks) rotates between
    projection outputs and score tiles; the accumulation pool (4 x
    [128, 512] = 4 banks) holds the per-head attn@v accumulators and the
    out-proj tiles.
"""

import numpy as np

D_MODEL = 1024
NUM_HEADS = 16
HEAD_DIM = 64
B = 2
S = 2048
N_CORES = 8
HEADS_PER_CORE = 4
C = HEADS_PER_CORE * HEAD_DIM  # 256 channels per core
SQ = 512                       # sq chunk (free dim of score matmuls)
NJ = S // SQ                   # 4 sq chunks
KT = 128                       # sk tile
NKT = S // KT                  # 16 sk tiles
NDT = D_MODEL // 128           # 8 contraction tiles for projections
UNROLL = 4                     # bodies per For_i iteration (barrier amortize)
# f16 Schraudolph exp: bitcast_f16(round(1024*(s*0.125*log2e + 15 - sigma)));
# sigma tuned for zero-mean multiplicative error (RMS ~1.8%); used for the
# hi=1 half of attention(3) non-diagonal tiles, where ScalarE is the
# bottleneck and DVE idles (softmax cancels the zero-mean error).
SCH_A = float(1024 * 1.4426950408889634 * 0.125)
SCH_B = float(1024 * (15 - 0.05640058203281127))

_CACHE = {}


def _build(loop_n=1, causal=True, unroll=False):
    import concourse.bass as bass
    import concourse.mybir as mybir
    import concourse.tile as tile
    from concourse import bacc

    dt = mybir.dt
    f16 = dt.float16
    f32 = dt.float32
    AF = mybir.ActivationFunctionType

    nc = bacc.Bacc(trn_type="TRN2", target_bir_lowering=False, debug=False)

    qT = nc.dram_tensor("qT", [D_MODEL, S], f16, kind="ExternalInput").ap()
    kT = nc.dram_tensor("kT", [D_MODEL, S], f16, kind="ExternalInput").ap()
    vT = nc.dram_tensor("vT", [D_MODEL, S], f16, kind="ExternalInput").ap()
    wq = nc.dram_tensor("wq", [D_MODEL, C], f16, kind="ExternalInput").ap()
    wk = nc.dram_tensor("wk", [D_MODEL, C], f16, kind="ExternalInput").ap()
    wv = nc.dram_tensor("wv", [D_MODEL, C], f16, kind="ExternalInput").ap()
    bq = nc.dram_tensor("bq", [128, 2], f32, kind="ExternalInput").ap()
    bk = nc.dram_tensor("bk", [128, 2], f32, kind="ExternalInput").ap()
    bv = nc.dram_tensor("bv", [1, C], f32, kind="ExternalInput").ap()
    wo = nc.dram_tensor("wo", [C, D_MODEL], f16, kind="ExternalInput").ap()
    masks = nc.dram_tensor("masks", [4, 128, 2 * SQ], f16, kind="ExternalInput").ap()
    out = nc.dram_tensor("out", [S, D_MODEL], f16, kind="ExternalOutput").ap()

    with tile.TileContext(nc) as tc:
        with tc.tile_pool(name="singles", bufs=1) as singles:
            # persistent SBUF tensors
            wq_sb = singles.tile([128, NDT, C], f16, tag="wq")
            wk_sb = singles.tile([128, NDT, C], f16, tag="wk")
            wv_sb = singles.tile([128, NDT, C], f16, tag="wv")
            wo_sb = singles.tile([128, 2, D_MODEL], f16, tag="wo")
            mask_sb = singles.tile([128, 4, 2 * SQ], f16, tag="mask")
            bq_sb = singles.tile([128, 2], f32, tag="bq")
            bk_sb = singles.tile([128, 2], f32, tag="bk")
            bv_sb = singles.tile([1, C], f32, tag="bv")
            bvbc_sb = singles.tile([128, C], f32, tag="bvbc")
            q_sb = singles.tile([128, 2, S], f16, tag="q")     # [c, s] 2 part tiles
            k_sb = singles.tile([128, 2, S], f16, tag="k")
            x_sb = singles.tile([128, 2, S], f16, tag="x")     # normalized attn out, [c, s]
            v_sb = singles.tile([128, NKT, HEADS_PER_CORE * 65], f16, tag="v")
            # staged chunk-0 inputs: written late in one For_i iteration,
            # read at the start of the next -- the all-engine barrier at the
            # For_i boundary otherwise forces a cold input-DMA wait.
            stg_sb = {nm: singles.tile([128, NDT, SQ], f16, tag=f"stg{nm}",
                                       name=f"stg_{nm}")
                      for nm in ("q", "k", "v")}

            # loop-invariant init: v ones-columns, weights, masks, biases
            nc.vector.memset(v_sb[:], 1.0)
            nc.sync.dma_start(out=bq_sb[:], in_=bq)
            nc.sync.dma_start(out=bk_sb[:], in_=bk)
            nc.sync.dma_start(out=bv_sb[:], in_=bv)
            nc.gpsimd.partition_broadcast(bvbc_sb[:], bv_sb[:])
            for ld in range(NDT):
                nc.sync.dma_start(out=wq_sb[:, ld, :], in_=wq[ld * 128:(ld + 1) * 128, :])
                nc.sync.dma_start(out=wk_sb[:, ld, :], in_=wk[ld * 128:(ld + 1) * 128, :])
                nc.sync.dma_start(out=wv_sb[:, ld, :], in_=wv[ld * 128:(ld + 1) * 128, :])
            for t in range(4):
                nc.sync.dma_start(out=mask_sb[:, t, :], in_=masks[t])
            for ct in range(2):
                nc.sync.dma_start(out=wo_sb[:, ct, :], in_=wo[ct * 128:(ct + 1) * 128, :])

            def stage_fetch():
                """Prefetch chunk 0 into the persistent staged tiles."""
                for nm, src in (("q", qT), ("k", kT), ("v", vT)):
                    for ld in range(NDT):
                        nc.sync.dma_start(
                            out=stg_sb[nm][:, ld, :],
                            in_=src[ld * 128:(ld + 1) * 128, 0:SQ],
                        )

            def staged_ink():
                return {nm: [stg_sb[nm][:, ld, :] for ld in range(NDT)]
                        for nm in ("q", "k", "v")}

            def body(inp, scp, accp, esb, nrm, osb,
                     use_staged=False, stage=False):
                def fetch_chunk(j):
                    """DMA the [128, 512] input tiles for chunk j; returns
                    {src: [8 tiles]}."""
                    sl = slice(j * SQ, (j + 1) * SQ)
                    tiles = {}
                    for nm, src in (("q", qT), ("k", kT), ("v", vT)):
                        tl = []
                        for ld in range(NDT):
                            it = inp.tile([128, SQ], f16, tag="ink")
                            nc.sync.dma_start(
                                out=it[:], in_=src[ld * 128:(ld + 1) * 128, sl]
                            )
                            tl.append(it)
                        tiles[nm] = tl
                    return tiles

                def proj_chunk(j, ink):
                    sl = slice(j * SQ, (j + 1) * SQ)
                    # q/k projections for chunk j -> qT_h/kT_h [c, s] slices
                    for nm, w_sb, b_sb, dest in (
                        ("q", wq_sb, bq_sb, q_sb),
                        ("k", wk_sb, bk_sb, k_sb),
                    ):
                        ps = scp.tile([128, 2 * SQ], f32, tag="sc")
                        for ld in range(NDT):
                            for ct in range(2):
                                nc.tensor.matmul(
                                    ps[:, ct * SQ:(ct + 1) * SQ],
                                    w_sb[:, ld, ct * 128:(ct + 1) * 128],
                                    ink[nm][ld][:],
                                    start=(ld == 0), stop=(ld == NDT - 1),
                                )
                        for ct in range(2):
                            nc.vector.tensor_scalar_add(
                                dest[:, ct, sl], ps[:, ct * SQ:(ct + 1) * SQ],
                                b_sb[:, ct:ct + 1],
                            )
                    # v projection for chunk j: natural [s, c], bias during evac
                    pv = scp.tile([128, 2 * SQ], f32, tag="sc")
                    for sti in range(4):
                        pvq = pv[:, sti * C:(sti + 1) * C]
                        for ld in range(NDT):
                            nc.tensor.matmul(
                                pvq, ink["v"][ld][:, sti * 128:(sti + 1) * 128],
                                wv_sb[:, ld, :], start=(ld == 0),
                                stop=(ld == NDT - 1),
                            )
                    for sti in range(4):
                        st = 4 * j + sti
                        vdst = v_sb[:, st, :].rearrange("p (h e) -> p h e", e=65)
                        nc.vector.tensor_add(
                            vdst[:, :, 0:64],
                            pv[:, sti * C:(sti + 1) * C].rearrange(
                                "p (h e) -> p h e", e=64),
                            bvbc_sb[:].rearrange("p (h e) -> p h e", e=64),
                        )

                def attention(j):
                    nkt = 4 * (j + 1) if causal else NKT

                    def sc_exp(hp, kt):
                        t = kt - 4 * j if (causal and kt >= 4 * j) else 0
                        diag = causal and kt >= 4 * j
                        psc = scp.tile([128, 2 * SQ], f32, tag="sc")
                        for hi in range(2):
                            lhsT = k_sb[64 * hi:64 * hi + 64, hp,
                                        kt * 128:(kt + 1) * 128]
                            rhs = q_sb[64 * hi:64 * hi + 64, hp,
                                       j * SQ + t * 128:(j + 1) * SQ]
                            nc.tensor.matmul(
                                psc[:, hi * SQ + t * 128:(hi + 1) * SQ],
                                lhsT, rhs, start=True, stop=True,
                            )
                        e2 = esb.tile([128, 2 * SQ], f16, tag="e2")
                        if causal and j == NJ - 1 and not diag:
                            nc.scalar.activation(e2[:, 0:SQ], psc[:, 0:SQ],
                                                 AF.Exp, scale=0.125)
                            nc.vector.tensor_scalar(
                                e2[:, SQ:2 * SQ].bitcast(dt.int16),
                                psc[:, SQ:2 * SQ], SCH_A, SCH_B,
                                mybir.AluOpType.mult, mybir.AluOpType.add,
                            )
                        elif diag:
                            # diagonal tile: columns < t*128 are fully masked
                            # -> narrowed; the 128-wide band at t*128 is
                            # triangular (exp + mask).
                            e3 = e2[:].rearrange("p (h c) -> p h c", h=2)
                            p3 = psc[:].rearrange("p (h c) -> p h c", h=2)
                            nc.scalar.activation(
                                e3[:, :, t * 128:SQ], p3[:, :, t * 128:SQ],
                                AF.Exp, scale=0.125,
                            )
                            m3 = mask_sb[:, t, :].rearrange(
                                "p (h c) -> p h c", h=2)
                            nc.vector.tensor_mul(
                                e3[:, :, t * 128:(t + 1) * 128],
                                e3[:, :, t * 128:(t + 1) * 128],
                                m3[:, :, t * 128:(t + 1) * 128],
                            )
                        else:
                            nc.scalar.activation(e2[:], psc[:], AF.Exp,
                                                 scale=0.125)
                        return e2, t

                    for hp in range(2):
                        po = {}
                        for hi in range(2):
                            po[hi] = accp.tile(
                                [128, SQ], f32, tag=f"acc{hi}", name=f"po{hp}{hi}"
                            )
                        # software-pipelined kt loop: scores/exp run one tile
                        # ahead of attn@v on the in-order PE queue.
                        pend = sc_exp(hp, 0)
                        for kt in range(nkt):
                            cur, pend = pend, (sc_exp(hp, kt + 1)
                                               if kt + 1 < nkt else None)
                            e2, t = cur
                            for hi in range(2):
                                h = 2 * hp + hi
                                lhsT = v_sb[:, kt, h * 65:(h + 1) * 65]
                                nc.tensor.matmul(
                                    po[hi][0:65, t * 128:SQ], lhsT,
                                    e2[:, hi * SQ + t * 128:(hi + 1) * SQ],
                                    start=(kt == 0), stop=(kt == nkt - 1),
                                )
                        for hi in range(2):
                            # D row -> SBUF (base partition 0), broadcast raw
                            # D (GpSimd), reciprocal on the broadcast, then
                            # scale during the PSUM->SBUF evacuation.
                            dsb = nrm.tile([1, SQ], f32, tag="dsb",
                                           name=f"d{hp}{hi}")
                            nc.vector.tensor_copy(dsb[:], po[hi][64:65, :])
                            dbc = nrm.tile([64, SQ], f32, tag="dbc",
                                           name=f"db{hp}{hi}")
                            nc.gpsimd.partition_broadcast(dbc[:], dsb[:])
                            rbc = nrm.tile([64, SQ], f32, tag="rbc",
                                           name=f"rb{hp}{hi}")
                            nc.vector.reciprocal_approx_fast(rbc[:], dbc[:])
                            nc.vector.tensor_mul(
                                x_sb[64 * hi:64 * hi + 64, hp,
                                     j * SQ:(j + 1) * SQ],
                                po[hi][0:64, :], rbc[:],
                            )

                def outproj(j, last=False):
                    for t in range(4 * j, 4 * j + 4):
                        pf = [accp.tile([128, SQ], f32, tag=f"acc{n}",
                                        name=f"pf{n}") for n in range(2)]
                        for ct in range(2):
                            lhsT = x_sb[:, ct, t * 128:(t + 1) * 128]
                            for n in range(2):
                                nc.tensor.matmul(
                                    pf[n][:], lhsT,
                                    wo_sb[:, ct, n * SQ:(n + 1) * SQ],
                                    start=(ct == 0), stop=(ct == 1),
                                )
                        ot = osb.tile([128, D_MODEL], f16, tag="ot")
                        for n in range(2):
                            nc.vector.tensor_copy(ot[:, n * SQ:(n + 1) * SQ],
                                                  pf[n][:])
                        # The last out-proj's DMAs issue from the Activation
                        # HWDGE queue (idle at body end) so they never
                        # head-of-line-block the next body's input prefetch
                        # on the SP queue.
                        eng = nc.scalar if last else nc.sync
                        eng.dma_start(out=out[t * 128:(t + 1) * 128, :],
                                      in_=ot[:])

                if causal:
                    ink = staged_ink() if use_staged else fetch_chunk(0)
                    for j in range(NJ):
                        proj_chunk(j, ink)
                        # prefetch chunk j+1 BEFORE the out-proj DMAs so input
                        # prefetch is never queued behind output traffic.
                        if j + 1 < NJ:
                            ink = fetch_chunk(j + 1)
                        if stage and j == 2:
                            # stage the NEXT body's chunk 0 now so its DMA
                            # completes before the For_i barrier.
                            stage_fetch()
                        if j > 0:
                            outproj(j - 1)
                        attention(j)
                    outproj(NJ - 1, last=True)
                else:
                    for j in range(NJ):
                        proj_chunk(j, fetch_chunk(j))
                    for j in range(NJ):
                        attention(j)
                        outproj(j, last=(j == NJ - 1))

            with (
                tc.tile_pool(name="inp", bufs=36) as inp,
                tc.tile_pool(name="sc", bufs=2, space="PSUM") as scp,
                tc.tile_pool(name="acc", bufs=2, space="PSUM") as accp,
                tc.tile_pool(name="esb", bufs=8) as esb,
                tc.tile_pool(name="nrm", bufs=8) as nrm,
                tc.tile_pool(name="osb", bufs=4) as osb,
            ):
                pools = (inp, scp, accp, esb, nrm, osb)
                if loop_n == 1:
                    body(*pools)
                elif unroll:
                    if causal:
                        stage_fetch()
                    for _ in range(loop_n):
                        body(*pools, use_staged=causal, stage=causal)
                else:
                    hint = (
                        mybir.EngineType.PE, mybir.EngineType.DVE,
                        mybir.EngineType.Activation, mybir.EngineType.Pool,
                        mybir.EngineType.SP,
                    )
                    nfull, rem = divmod(loop_n, UNROLL)
                    if causal:
                        stage_fetch()
                    if nfull:
                        with tc.For_i(0, nfull, 1, hint_engines=hint):
                            for u in range(UNROLL):
                                body(*pools,
                                     use_staged=(causal and u == 0),
                                     stage=(causal and u == UNROLL - 1))
                    for r in range(rem):
                        body(*pools, use_staged=(causal and r == 0))

    nc.compile()
    return nc


def _masks_np():
    m = np.zeros((4, 128, 2 * SQ), np.float16)
    r = np.arange(128)[:, None]
    c = np.arange(SQ)[None, :]
    for t in range(4):
        allow = (t * 128 + r) <= c
        m[t, :, 0:SQ] = allow
        m[t, :, SQ:2 * SQ] = allow
    return m


def _in_maps(query, key, value, w_q, b_q, w_k, b_k, w_v, b_v, w_o):
    f16 = np.float16
    masks = _masks_np()
    per_batch = []
    for b in range(B):
        per_batch.append((
            query[b].T.astype(f16),
            key[b].T.astype(f16),
            value[b].T.astype(f16),
        ))
    in_maps = []
    for core in range(N_CORES):
        b, g = divmod(core, N_CORES // B)
        cols = slice(g * C, (g + 1) * C)
        qTb, kTb, vTb = per_batch[b]
        in_maps.append({
            "qT": qTb, "kT": kTb, "vT": vTb,
            "wq": w_q[:, cols].astype(f16),
            "wk": w_k[:, cols].astype(f16),
            "wv": w_v[:, cols].astype(f16),
            # [128, 2] layout: bq2[p, ct] = b_q[g*C + ct*128 + p]
            "bq": b_q[cols].reshape(2, 128).T.copy().astype(np.float32),
            "bk": b_k[cols].reshape(2, 128).T.copy().astype(np.float32),
            "bv": b_v[cols].reshape(1, -1).astype(np.float32),
            "wo": np.ascontiguousarray(w_o[cols, :]).astype(f16),
            "masks": masks,
        })
    return in_maps


def kernel(query, key, value, mask, w_q, b_q, w_k, b_k, w_v, b_v, w_o, b_o):
    from concourse.bass_utils import run_bass_kernel_spmd

    query = np.asarray(query, np.float32)
    key = np.asarray(key, np.float32)
    value = np.asarray(value, np.float32)
    causal = bool(np.asarray(mask).reshape(-1)[0]) if np.asarray(mask).size else False

    ck = ("prog", causal)
    if ck not in _CACHE:
        _CACHE[ck] = _build(loop_n=1, causal=causal)
    nc = _CACHE[ck]

    in_maps = _in_maps(query, key, value,
                       np.asarray(w_q, np.float32), np.asarray(b_q, np.float32),
                       np.asarray(w_k, np.float32), np.asarray(b_k, np.float32),
                       np.asarray(w_v, np.float32), np.asarray(b_v, np.float32),
                       np.asarray(w_o, np.float32))
    res = run_bass_kernel_spmd(nc, in_maps, core_ids=list(range(N_CORES)))
    outs = [res.results[c]["out"] for c in range(N_CORES)]
    gpb = N_CORES // B
    full = np.empty((B, S, D_MODEL), np.float32)
    bo = np.asarray(b_o, np.float32)
    for b in range(B):
        acc = outs[gpb * b].astype(np.float32)
        for g in range(1, gpb):
            acc += outs[gpb * b + g].astype(np.float32)
        full[b] = acc + bo[None, :]
    return full
